# revision 22
# baseline (speedup 1.0000x reference)
"""Multi-head attention (B=4, S=2048, D=1024, H=16) on 8 trn2 NeuronCores.

Sharding: (batch, head-group) -> 8 shards of (1 batch x 8 heads). Zero
cross-core communication: each core computes Q/K/V projections for its 8
heads, full attention over S=2048, and a partial output projection
(row-split Wo); the host sums the two head-group partials per batch.

Layout strategy: the host feeds per-batch inputs pre-transposed and
pre-tiled so every device DMA is a contiguous partition-major block
(trivial descriptor generation on the sync engine). Everything stays in
"feature-major" form on device:
  Q^T, K^T: [dh, s]  -> scores^T[sk, q]  (softmax along partitions is
  avoided via an appended ones-column on V, which makes the PV matmul emit
  the softmax denominator as an extra output row)
  V: natural [s, dh] (+ ones col) -> ctx^T[dh(+1), q]
  out^T[do, q] = Wo_slice^T-contract(ctx^T / rowsum)

Key optimizations vs the 441us/444us baseline:
  - software-pipelined attention inner loop: per iteration emit
    scores(k+1), exp(k+1), PV(k). The PV matmul pair's LDWEIGHTS (which
    carries the matmul's semaphore waits after bacc's
    move_matmul_waits_to_ldweights pass) then waits on an exp that
    completed a full iteration earlier, so the V/K^T stationary loads
    issue early and background-load under the previous matmul's
    streaming instead of stalling the exp-paced critical path
    (~+118-160ns x 512 swaps in the baseline trace).
  - the PV lag also gives the ctx PSUM bank a 2-slot window for its
    copy-out before the next head's PV needs the bank (ps_ctx bufs=1).
  - all host->device tensors re-laid-out so each DMA is contiguous per
    partition (baseline DIRECT2D issue was 3-5us per strided DMA on the
    serial sync sequencer; first matmul waited until t=22.7us).
  - emission order k0,q0,q1 before v0 so the first exp's inputs
    (KT sb0 + QT qp0) are ready ~30us earlier.
  - o-projection emitted OUTSIDE the high-priority attention stream:
    o-proj(qp0) backfills attention(qp1)'s tensor slack instead of
    blocking it (baseline had a 13.5us exp gap at the qp transition).

Matmul operands are fp16 (fp32 PSUM accumulation; the PE multiplies at
FP22 internally). fp16 supports fast-weight-load + background-buffer
overlap. Output partials DMA'd as fp16 (host sums in fp32).

Known dead ends (measured in prior sessions): fp8 DoubleRow (3-5% error
>> 2e-2 budget), PV flipped to M=128 (per-65-col LDWEIGHTS exceeds
column savings), >512-col matmuls (PSUM bank crossing), tile_critical
LDW dedup (drain overhead), narrow (<1024) exps (per-instr ACT overhead).
"""

import numpy as np

import concourse.bass as bass
import concourse.tile as tile
from concourse import bacc, mybir
from concourse.bass_utils import run_bass_kernel_spmd

F32 = mybir.dt.float32
F16 = mybir.dt.float16
AF = mybir.ActivationFunctionType

B, S, D = 4, 2048, 1024
HPC = 8          # heads per core
DHT = 512        # head dims per core (8 * 64)
NDT = D // 128   # 8 d-tiles (contraction tiles for projections)
NHT = DHT // 128  # 4 dh-tiles
NST = S // 128   # 16 s-tiles
NSB = S // 512   # 4 s-blocks
N_CORES = 8


def build_nc():
    nc = bacc.Bacc(None, target_bir_lowering=False)

    # All DRAM layouts are pre-tiled on host so every DMA is contiguous
    # per partition.
    xq = nc.declare_dram_parameter("xq_t", [NSB, 128, NDT, 512], F16, isOutput=False)
    xk = nc.declare_dram_parameter("xk_t", [NSB, 128, NDT, 512], F16, isOutput=False)
    xv = nc.declare_dram_parameter("xv_t", [NSB, 128, NDT, 512], F16, isOutput=False)
    wq = nc.declare_dram_parameter("wq", [128, NDT, DHT], F16, isOutput=False)
    wk = nc.declare_dram_parameter("wk", [128, NDT, DHT], F16, isOutput=False)
    wv = nc.declare_dram_parameter("wv", [128, NDT, DHT], F16, isOutput=False)
    wo = nc.declare_dram_parameter("wo", [128, NHT, D], F16, isOutput=False)
    bq = nc.declare_dram_parameter("bq", [128, NHT], F32, isOutput=False)
    bk = nc.declare_dram_parameter("bk", [128, NHT], F32, isOutput=False)
    bv = nc.declare_dram_parameter("bv", [DHT], F32, isOutput=False)
    # output: [kt-partial, q-block 512, dot, p, c] -> host sums kt
    # partials and reassembles
    ot = nc.declare_dram_parameter("o_t", [NHT, 4, NDT, 128, 512], F16, isOutput=True)

    with tile.TileContext(nc) as tc:
        with (
            tc.tile_pool(name="persist", bufs=1) as persist,
            tc.tile_pool(name="outp", bufs=4) as outp,
            tc.tile_pool(name="w3", bufs=1) as w3,
            tc.tile_pool(name="xs", bufs=5) as xs,
            tc.tile_pool(name="pexp_p", bufs=4) as pexp_p,
            tc.tile_pool(name="small", bufs=2) as small,
            tc.tile_pool(name="ps_big", bufs=2, space="PSUM") as ps_big,
            tc.tile_pool(name="ps_ctx", bufs=1, space="PSUM") as ps_ctx,
            tc.tile_pool(name="ps_o", bufs=2, space="PSUM") as ps_o,
        ):
            # KT/Vt padded to full (128,128) stationaries: only full-
            # square weight loads background-load under the previous
            # matmul's streaming (FWL); (64,128)/(128,65) stationaries
            # measured +102/+159ns per swap. Pad rows/cols are zeros.
            KT = persist.tile([128, HPC, S], F16)        # K^T per head, half zero
            QT = persist.tile([128, NHT, S], F16)        # Q^T  [dh, s]
            Vt = persist.tile([128, NST, HPC, 128], F16)  # V + ones col + zero pad
            ctxn = persist.tile([128, NHT, S], F16)      # normalized ctx^T
            wo_sb = persist.tile([128, NHT, D], F16)
            bq_sb = persist.tile([128, NHT], F32)
            bk_sb = persist.tile([128, NHT], F32)
            bv_bc = persist.tile([128, HPC, 64], F32)
            wq_sb = w3.tile([128, NDT, DHT], F16)
            wk_sb = w3.tile([128, NDT, DHT], F16)
            wv_sb = w3.tile([128, NDT, DHT], F16)

            # critical-path DMAs first (first k-proj needs them)
            nc.sync.dma_start(out=wk_sb, in_=wk.ap())
            nc.sync.dma_start(out=bq_sb, in_=bq.ap())
            nc.sync.dma_start(out=bk_sb, in_=bk.ap())
            nc.sync.dma_start(
                out=bv_bc,
                in_=bv.rearrange("(h d) -> h d", d=64).partition_broadcast(128),
            )
            # ones column for the PV matmul's softmax-denominator row.
            # Vt's pad cols 65:127 stay uninitialized: they only produce
            # ctx rows 65:127, which are never read.
            nc.vector.memset(Vt[:, :, :, 64:65], 1.0)
            # KT pad halves must be exact zeros (they multiply the other
            # head's QT rows). Emitted per head on the otherwise-idle
            # gpsimd engine so the DVE queue stays free for bias-adds.
            for h in range(HPC):
                pad0 = 64 - 64 * (h % 2)
                nc.gpsimd.memset(KT[pad0 : pad0 + 64, h, :], 0.0)

            xst_cache = {}

            def emit_proj(kind, sb, hts=tuple(range(NHT))):
                xp = {"k": xk, "v": xv, "q": xq}[kind]
                if (kind, sb) in xst_cache:
                    xst = xst_cache[(kind, sb)]
                else:
                    xst = xs.tile([128, NDT, 512], F16, tag="xs")
                    nc.sync.dma_start(out=xst, in_=xp.ap()[sb])
                    xst_cache[(kind, sb)] = xst
                if kind == "v":
                    # V projection, natural layout: the X^T tile is
                    # stationary so out[s-tile, dh] has s on partitions
                    for su in range(4):
                        pso = ps_o.tile([128, 512], F32, tag="po")
                        for dt in range(NDT):
                            nc.tensor.matmul(
                                pso[:, :],
                                xst[:, dt, bass.ts(su, 128)],
                                wv_sb[:, dt, :],
                                start=(dt == 0),
                                stop=(dt == NDT - 1),
                            )
                        nc.vector.tensor_add(
                            out=Vt[:, sb * 4 + su, :, 0:64],
                            in0=pso.rearrange("p (h d) -> p h d", d=64),
                            in1=bv_bc,
                        )
                else:
                    ssl = slice(sb * 512, (sb + 1) * 512)
                    w_sb = wk_sb if kind == "k" else wq_sb
                    b_sb = bk_sb if kind == "k" else bq_sb
                    # K^T / Q^T: out[dh-tile, s-blk] = W^T-contract X^T
                    for ht in hts:
                        ps = ps_o.tile([128, 512], F32, tag="po")
                        for dt in range(NDT):
                            nc.tensor.matmul(
                                ps[:, :],
                                w_sb[:, dt, bass.ts(ht, 128)],
                                xst[:, dt, :],
                                start=(dt == 0),
                                stop=(dt == NDT - 1),
                            )
                        # DVE (not ACT) so the scalar engine stays
                        # free for the softmax exps
                        if kind == "q":
                            nc.vector.tensor_scalar_add(
                                out=QT[:, ht, ssl],
                                in0=ps[:, :],
                                scalar1=b_sb[:, ht : ht + 1],
                            )
                        else:
                            # scatter the head pair into per-head padded
                            # planes (other 64 rows stay zero)
                            for hh in range(2):
                                po = 64 * hh
                                nc.vector.tensor_scalar_add(
                                    out=KT[po : po + 64, 2 * ht + hh, ssl],
                                    in0=ps[po : po + 64, :],
                                    scalar1=b_sb[po : po + 64, ht : ht + 1],
                                )

            def emit_scores(qp, h, sk):
                q0 = qp * 1024
                po = 64 * (h % 2)
                ht = h // 2
                st = ps_big.tile([128, 1024], F32, tag="big")
                for j in range(2):
                    nc.tensor.matmul(
                        st[:, bass.ts(j, 512)],
                        KT[:, h, bass.ts(sk, 128)],
                        QT[:, ht, q0 + j * 512 : q0 + (j + 1) * 512],
                        start=True,
                        stop=True,
                    )
                pexp = pexp_p.tile([128, 1024], F16, tag="pexp")
                nc.scalar.activation(out=pexp, in_=st, func=AF.Exp, scale=0.125)
                return pexp

            def emit_pv(h, sk, pexp, ctx):
                for j in range(2):
                    nc.tensor.matmul(
                        ctx[:, bass.ts(j, 512)],
                        Vt[:, sk, h, :],
                        pexp[:, bass.ts(j, 512)],
                        start=(sk == 0),
                        stop=(sk == NST - 1),
                    )

            def emit_normalize(qp, h, ctx):
                # copy PSUM out fast to release the ctx bank for the
                # next head; normalize from SBUF off the critical path.
                # The sums row is DMA-reshaped across 128 partitions so
                # the (8 cyc/elem) reciprocal runs 128-wide.
                q0 = qp * 1024
                po = 64 * (h % 2)
                ht = h // 2
                ctxc = small.tile([65, 1024], F32, tag="ctxc")
                nc.vector.tensor_copy(out=ctxc, in_=ctx[0:65, :])
                rr = small.tile([128, 8], F32, tag="rr")
                nc.sync.dma_start(out=rr, in_=ctxc[64:65, :])
                rrv = small.tile([128, 8], F32, tag="rrv")
                nc.vector.reciprocal(out=rrv, in_=rr)
                rinvrow = small.tile([1, 1024], F32, tag="rinvrow")
                nc.sync.dma_start(out=rinvrow, in_=rrv)
                rbc = small.tile([64, 1024], F32, tag="rbc")
                nc.gpsimd.partition_broadcast(rbc, rinvrow)
                nc.vector.tensor_mul(
                    out=ctxn[po : po + 64, ht, q0 : q0 + 1024],
                    in0=ctxc[0:64, :],
                    in1=rbc,
                )

            def emit_oproj(qp):
                # per-kt PARTIAL output projections: each (dot, j, kt) is
                # a single matmul whose ctxn[kt] slice is ready as soon as
                # head pair 2kt/2kt+1 finishes, so o-proj work backfills
                # the attention phase instead of serializing at the end.
                # The host sums the 4 kt-partials (and the 2 head-group
                # partials) in fp32.
                q0 = qp * 1024
                for kt in range(NHT):
                    for dot in range(8):
                        for j in range(2):
                            pso = ps_o.tile([128, 512], F32, tag="po")
                            nc.tensor.matmul(
                                pso[:, :],
                                wo_sb[:, kt, bass.ts(dot, 128)],
                                ctxn[:, kt, q0 + j * 512 : q0 + (j + 1) * 512],
                                start=True,
                                stop=True,
                            )
                            osb = outp.tile([128, 512], F16, tag="osb")
                            # the final head-tile's partials are the
                            # kernel tail: route their PSUM->SBUF copies
                            # through the scalar engine (idle once the
                            # exps are done) so the tail isn't DVE-serial
                            if qp == 1 and kt == NHT - 1 and (dot + j) % 2 == 0:
                                nc.scalar.copy(out=osb, in_=pso)
                            else:
                                nc.vector.tensor_copy(out=osb, in_=pso)
                            nc.sync.dma_start(
                                out=ot.ap()[kt, qp * 2 + j, dot], in_=osb
                            )

            # Projections, emitted in first-need order (program order =
            # backfill tiebreak order for the scheduler). The attention
            # stream below runs at high priority and preempts as soon as
            # each of its inputs is placed; remaining projection matmuls
            # fill the PE slack of the exp-paced attention phase.
            emit_proj("k", 0)
            nc.sync.dma_start(out=wq_sb, in_=wq.ap())
            emit_proj("q", 0, hts=(0,))
            emit_proj("q", 1, hts=(0,))
            nc.sync.dma_start(out=wv_sb, in_=wv.ap())
            emit_proj("v", 0)
            # Emission position must satisfy the xs-pool WAR rule (tile
            # #N's DMA waits tile #(N-bufs)'s readers, and the sync queue
            # is in-order), so the K x-blocks are allocated early and the
            # q.ht123 filler is emitted before k3/v1 whose buffers reuse
            # q0/q1's. Priority boosts make the scheduler still run the
            # urgent work (k.ht0 feeds the exp chain directly, v feeds PV
            # pexp-lag behind) ahead of the filler.
            with tc.high_priority(offset=150):
                emit_proj("k", 1, hts=(0,))
                emit_proj("k", 2, hts=(0,))
            emit_proj("q", 0, hts=(1, 2, 3))
            emit_proj("q", 1, hts=(1, 2, 3))
            with tc.high_priority(offset=150):
                emit_proj("k", 3, hts=(0,))
                emit_proj("v", 1)
            emit_proj("k", 1, hts=(1, 2, 3))
            with tc.high_priority(offset=150):
                emit_proj("v", 2)
            emit_proj("k", 2, hts=(1, 2, 3))
            with tc.high_priority(offset=150):
                emit_proj("v", 3)
            emit_proj("k", 3, hts=(1, 2, 3))
            nc.sync.dma_start(out=wo_sb, in_=wo.ap())
            emit_proj("q", 2)
            emit_proj("q", 3)

            # Attention, software-pipelined with PV lagging one iteration:
            # per slot emit scores(k+1), exp(k+1), then PV(k). The PV
            # LDWEIGHTS' waits (inherited from the matmul) are satisfied a
            # full slot early, so stationary swaps background-load under
            # the previous matmuls instead of stalling.
            iters = [
                (qp, h, sk)
                for qp in (0, 1)
                for h in range(HPC)
                for sk in range(NST)
            ]
            # PV lags its exp by TWO iterations: when the PE sequencer
            # reaches the PV pair (and the LDW/EventSemaphore carrying
            # its moved waits), the exp completed ~2.2us earlier, so the
            # sequencer flows through without stalling and the V weight
            # load hides under the preceding matmuls. (With lag-1 the
            # wait resolves exactly when the load is needed: +159ns on
            # every iteration's first PV matmul, measured.)
            LAG = 2
            with tc.high_priority():
                pendq = []  # [(qp, h, sk, pexp)]
                ctx_cur = None

                def emit_pend():
                    # PV for the oldest pending slot; allocates the
                    # head's ctx accumulator at its first write so the
                    # ps_ctx rotation dep lands after the previous
                    # head's copy-out emission.
                    nonlocal ctx_cur
                    pqp, ph, psk, ppexp = pendq.pop(0)
                    if psk == 0:
                        ctx_cur = ps_ctx.tile([128, 1024], F32, tag="ctx")
                    emit_pv(ph, psk, ppexp, ctx_cur)
                    if psk == NST - 1:
                        emit_normalize(pqp, ph, ctx_cur)

                for qp, h, sk in iters:
                    pexp = emit_scores(qp, h, sk)
                    pendq.append((qp, h, sk, pexp))
                    if len(pendq) > LAG:
                        emit_pend()
                while pendq:
                    emit_pend()

            # o-projections at normal priority: o-proj(qp0) backfills
            # attention(qp1)'s tensor slack; o-proj(qp1) is the tail.
            emit_oproj(0)
            emit_oproj(1)

    nc.compile()
    return nc


_NC_CACHE = None


def _get_nc():
    global _NC_CACHE
    if _NC_CACHE is None:
        _NC_CACHE = build_nc()
    return _NC_CACHE


def make_in_maps(q, k, v, Wq, bq, Wk, bk, Wv, bv, Wo):
    bf = np.float16

    def x_tiles(x):
        # [S, D] -> [sb, p, t, c] with s = sb*512 + c, d = t*128 + p
        return np.ascontiguousarray(
            x.reshape(NSB, 512, NDT, 128).transpose(0, 3, 2, 1)
        ).astype(bf)

    def w_tiles(w):
        # [D, DHT] -> [p, t, n] with d = t*128 + p
        return np.ascontiguousarray(
            w.reshape(NDT, 128, DHT).transpose(1, 0, 2)
        ).astype(bf)

    in_maps = []
    for core in range(N_CORES):
        b, hg = core // 2, core % 2
        csl = slice(hg * DHT, (hg + 1) * DHT)
        in_maps.append(
            {
                "xq_t": x_tiles(q[b]),
                "xk_t": x_tiles(k[b]),
                "xv_t": x_tiles(v[b]),
                "wq": w_tiles(Wq[:, csl]),
                "wk": w_tiles(Wk[:, csl]),
                "wv": w_tiles(Wv[:, csl]),
                "wo": np.ascontiguousarray(
                    Wo[csl, :].reshape(NHT, 128, D).transpose(1, 0, 2)
                ).astype(bf),
                "bq": np.ascontiguousarray(
                    bq[csl].reshape(NHT, 128).T
                ).astype(np.float32),
                "bk": np.ascontiguousarray(
                    bk[csl].reshape(NHT, 128).T
                ).astype(np.float32),
                "bv": np.ascontiguousarray(bv[csl]).astype(np.float32),
            }
        )
    return in_maps


def kernel(q, k, v, Wq, bq, Wk, bk, Wv, bv, Wo, bo):
    q = np.asarray(q, np.float32)
    k = np.asarray(k, np.float32)
    v = np.asarray(v, np.float32)
    Wq = np.asarray(Wq, np.float32)
    Wk = np.asarray(Wk, np.float32)
    Wv = np.asarray(Wv, np.float32)
    Wo = np.asarray(Wo, np.float32)
    bq = np.asarray(bq, np.float32)
    bk = np.asarray(bk, np.float32)
    bv = np.asarray(bv, np.float32)
    bo = np.asarray(bo, np.float32)

    nc = _get_nc()
    in_maps = make_in_maps(q, k, v, Wq, bq, Wk, bk, Wv, bv, Wo)
    res = run_bass_kernel_spmd(nc, in_maps, list(range(N_CORES)))
    out = np.empty((B, S, D), np.float32)
    for b in range(B):
        # o_t [kt, qb, dot, p, c]: sum kt partials (and the two
        # head-group cores), then [s, d] with s = qb*512+c, d = dot*128+p
        o_t = res.results[2 * b]["o_t"].astype(np.float32).sum(axis=0) + res.results[
            2 * b + 1
        ]["o_t"].astype(np.float32).sum(axis=0)
        out[b] = o_t.transpose(0, 3, 1, 2).reshape(S, D) + bo
    return out


# revision 24
# speedup vs baseline: 1.0101x; 1.0101x over previous
"""Multi-head attention (B=4, S=2048, D=1024, H=16) on 8 trn2 NeuronCores.

Sharding: (batch, head-group) -> 8 shards of (1 batch x 8 heads). Zero
cross-core communication: each core computes Q/K/V projections for its 8
heads, full attention over S=2048, and a partial output projection
(row-split Wo); the host sums the head-group and kt partials per batch.

Layout strategy: the host feeds per-batch inputs pre-transposed and
pre-tiled so every device DMA is a contiguous partition-major block
(trivial descriptor generation on the serial sync sequencer; strided
DMAs cost 3-5us each to issue). On device, everything stays in
"feature-major" form:
  Q^T, K^T: [dh, s]  -> scores^T[sk, q]  (softmax along partitions is
  avoided via an appended ones-column on V, which makes the PV matmul
  emit the softmax denominator as an extra output row)
  V: natural [s, dh] (+ ones col) -> ctx^T[dh(+1), q]
  out^T partials: per kt, wo[kt]^T-contract(ctx^T[kt] / rowsum)

Optimizations vs the 444.3us baseline (this config: ~397.4us):
  - KT/V stationaries padded to full (128,128): the PE only background-
    loads weights under the previous matmul's streaming for full-square
    fp16 stationaries (FWL); (64,128)/(128,65) pay +102/+159ns per swap
    (trace-measured). KT pad rows are exact zeros (memset per head on
    the idle gpsimd engine; a monolithic DVE memset runs 13.7us and
    blocks the in-order DVE queue); V pad cols stay uninitialized (they
    only feed ctx rows 65:127, which are never read).
  - software-pipelined attention with PV lagging its exp by TWO
    iterations: bacc's move_matmul_waits_to_ldweights /
    generate_event_semaphores place secondary waits on the LDWEIGHTS or
    an EventSemaphore ahead of the matmul, and the in-order PE sequencer
    stalls there, blocking weight prefetch. With lag-2 every wait is
    satisfied ~2.2us before the sequencer reaches it (lag-1 is not
    enough: the saturated scalar queue means exp(k) ends exactly when
    PV(k) wants to start). The lag also gives the single-buffered ctx
    PSUM bank a 2-slot window for its copy-out.
  - per-kt PARTIAL output projections (single-matmul PSUM groups, host
    sums the 4 kt partials in fp32): each partial only needs one
    head-pair's ctxn, so o-proj backfills the attention phase; the tail
    after the last exp is just the last head-pair's 16 partials, whose
    PSUM->SBUF copies alternate scalar/DVE engines.
  - emission in first-consumption order (k0, q0.ht0, q1.ht0, v0, then
    per-sb k.ht0 [priority-boosted] / v / k.ht123, then q late blocks):
    program order is the scheduler's backfill tiebreak, and the exp
    chain consumes K in sb order. The early phase is projection-
    throughput-bound (h0 needs all of K and V before it completes).
  - emission-position constraints: the sync queue is in-order and an
    xs-pool tile's DMA waits on the readers of the tile bufs-ago, so
    every consumer of that earlier tile must be emitted before the
    DMA's position or the Tile scheduler deadlocks.

Matmul operands are fp16 (fp32 PSUM accumulation; PE multiplies at FP22
internally). Softmax exp on the scalar engine (1024-wide; narrower exps
lose to the ~230ns/instr ACT overhead). Engine floors per core: tensor
786432 streamed columns = 327.7us @2.4GHz, scalar 256 exps ~268us; the
kernel is tensor-bound at ~356us busy + ~19us DMA-gated startup + tail.

Known dead ends (measured): fp8 DoubleRow (3-5% error >> 2e-2 budget),
PV flipped to M=128, >512-col matmuls (PSUM bank crossing), paired-head
row-tiled scores (needs 10+ PSUM banks, kills projection backfill),
DRAM-bounce broadcast for the reciprocal row (DRAM roundtrip latency),
f32 output partials (DMA doubles, outp rotation throttles the tail).
"""

import numpy as np

import concourse.bass as bass
import concourse.tile as tile
from concourse import bacc, mybir
from concourse.bass_utils import run_bass_kernel_spmd

F32 = mybir.dt.float32
F16 = mybir.dt.float16
AF = mybir.ActivationFunctionType

B, S, D = 4, 2048, 1024
HPC = 8          # heads per core
DHT = 512        # head dims per core (8 * 64)
NDT = D // 128   # 8 d-tiles (contraction tiles for projections)
NHT = DHT // 128  # 4 dh-tiles
NST = S // 128   # 16 s-tiles
NSB = S // 512   # 4 s-blocks
N_CORES = 8


def build_nc():
    nc = bacc.Bacc(None, target_bir_lowering=False)

    # All DRAM layouts are pre-tiled on host so every DMA is contiguous
    # per partition.
    xq = nc.declare_dram_parameter("xq_t", [NSB, 128, NDT, 512], F16, isOutput=False)
    xk = nc.declare_dram_parameter("xk_t", [NSB, 128, NDT, 512], F16, isOutput=False)
    xv = nc.declare_dram_parameter("xv_t", [NSB, 128, NDT, 512], F16, isOutput=False)
    wq = nc.declare_dram_parameter("wq", [128, NDT, DHT], F16, isOutput=False)
    wk = nc.declare_dram_parameter("wk", [128, NDT, DHT], F16, isOutput=False)
    wv = nc.declare_dram_parameter("wv", [128, NDT, DHT], F16, isOutput=False)
    wo = nc.declare_dram_parameter("wo", [128, NHT, D], F16, isOutput=False)
    bq = nc.declare_dram_parameter("bq", [128, NHT], F32, isOutput=False)
    bk = nc.declare_dram_parameter("bk", [128, NHT], F32, isOutput=False)
    bv = nc.declare_dram_parameter("bv", [DHT], F32, isOutput=False)
    # output: [kt-partial, q-block 512, dot, p, c] -> host sums kt
    # partials and reassembles
    ot = nc.declare_dram_parameter("o_t", [NHT, 4, NDT, 128, 512], F16, isOutput=True)

    with tile.TileContext(nc) as tc:
        with (
            tc.tile_pool(name="persist", bufs=1) as persist,
            tc.tile_pool(name="outp", bufs=4) as outp,
            tc.tile_pool(name="w3", bufs=1) as w3,
            tc.tile_pool(name="xs", bufs=5) as xs,
            tc.tile_pool(name="pexp_p", bufs=4) as pexp_p,
            tc.tile_pool(name="small", bufs=2) as small,
            tc.tile_pool(name="ps_big", bufs=2, space="PSUM") as ps_big,
            tc.tile_pool(name="ps_ctx", bufs=1, space="PSUM") as ps_ctx,
            tc.tile_pool(name="ps_o", bufs=2, space="PSUM") as ps_o,
        ):
            # KT/Vt padded to full (128,128) stationaries: only full-
            # square weight loads background-load under the previous
            # matmul's streaming (FWL); (64,128)/(128,65) stationaries
            # measured +102/+159ns per swap. Pad rows/cols are zeros.
            KT = persist.tile([128, HPC, S], F16)        # K^T per head, half zero
            QT = persist.tile([128, NHT, S], F16)        # Q^T  [dh, s]
            Vt = persist.tile([128, NST, HPC, 128], F16)  # V + ones col + zero pad
            ctxn = persist.tile([128, NHT, S], F16)      # normalized ctx^T
            wo_sb = persist.tile([128, NHT, D], F16)
            bq_sb = persist.tile([128, NHT], F32)
            bk_sb = persist.tile([128, NHT], F32)
            bv_bc = persist.tile([128, HPC, 64], F32)
            wq_sb = w3.tile([128, NDT, DHT], F16)
            wk_sb = w3.tile([128, NDT, DHT], F16)
            wv_sb = w3.tile([128, NDT, DHT], F16)

            # critical-path DMAs first (first k-proj needs them)
            nc.sync.dma_start(out=wk_sb, in_=wk.ap())
            nc.sync.dma_start(out=bq_sb, in_=bq.ap())
            nc.sync.dma_start(out=bk_sb, in_=bk.ap())
            nc.sync.dma_start(
                out=bv_bc,
                in_=bv.rearrange("(h d) -> h d", d=64).partition_broadcast(128),
            )
            # ones column for the PV matmul's softmax-denominator row.
            # Vt's pad cols 65:127 stay uninitialized: they only produce
            # ctx rows 65:127, which are never read.
            nc.vector.memset(Vt[:, :, :, 64:65], 1.0)
            # KT pad halves must be exact zeros (they multiply the other
            # head's QT rows). Emitted per head on the otherwise-idle
            # gpsimd engine so the DVE queue stays free for bias-adds.
            for h in range(HPC):
                pad0 = 64 - 64 * (h % 2)
                nc.gpsimd.memset(KT[pad0 : pad0 + 64, h, :], 0.0)

            xst_cache = {}

            def emit_proj(kind, sb, hts=tuple(range(NHT))):
                xp = {"k": xk, "v": xv, "q": xq}[kind]
                if (kind, sb) in xst_cache:
                    xst = xst_cache[(kind, sb)]
                else:
                    xst = xs.tile([128, NDT, 512], F16, tag="xs")
                    nc.sync.dma_start(out=xst, in_=xp.ap()[sb])
                    xst_cache[(kind, sb)] = xst
                if kind == "v":
                    # V projection, natural layout: the X^T tile is
                    # stationary so out[s-tile, dh] has s on partitions
                    for su in range(4):
                        pso = ps_o.tile([128, 512], F32, tag="po")
                        for dt in range(NDT):
                            nc.tensor.matmul(
                                pso[:, :],
                                xst[:, dt, bass.ts(su, 128)],
                                wv_sb[:, dt, :],
                                start=(dt == 0),
                                stop=(dt == NDT - 1),
                            )
                        nc.vector.tensor_add(
                            out=Vt[:, sb * 4 + su, :, 0:64],
                            in0=pso.rearrange("p (h d) -> p h d", d=64),
                            in1=bv_bc,
                        )
                else:
                    ssl = slice(sb * 512, (sb + 1) * 512)
                    w_sb = wk_sb if kind == "k" else wq_sb
                    b_sb = bk_sb if kind == "k" else bq_sb
                    # K^T / Q^T: out[dh-tile, s-blk] = W^T-contract X^T
                    for ht in hts:
                        ps = ps_o.tile([128, 512], F32, tag="po")
                        for dt in range(NDT):
                            nc.tensor.matmul(
                                ps[:, :],
                                w_sb[:, dt, bass.ts(ht, 128)],
                                xst[:, dt, :],
                                start=(dt == 0),
                                stop=(dt == NDT - 1),
                            )
                        # DVE (not ACT) so the scalar engine stays
                        # free for the softmax exps
                        if kind == "q":
                            nc.vector.tensor_scalar_add(
                                out=QT[:, ht, ssl],
                                in0=ps[:, :],
                                scalar1=b_sb[:, ht : ht + 1],
                            )
                        else:
                            # scatter the head pair into per-head padded
                            # planes (other 64 rows stay zero)
                            for hh in range(2):
                                po = 64 * hh
                                nc.vector.tensor_scalar_add(
                                    out=KT[po : po + 64, 2 * ht + hh, ssl],
                                    in0=ps[po : po + 64, :],
                                    scalar1=b_sb[po : po + 64, ht : ht + 1],
                                )

            def emit_scores(qp, h, sk):
                q0 = qp * 1024
                po = 64 * (h % 2)
                ht = h // 2
                st = ps_big.tile([128, 1024], F32, tag="big")
                for j in range(2):
                    nc.tensor.matmul(
                        st[:, bass.ts(j, 512)],
                        KT[:, h, bass.ts(sk, 128)],
                        QT[:, ht, q0 + j * 512 : q0 + (j + 1) * 512],
                        start=True,
                        stop=True,
                    )
                pexp = pexp_p.tile([128, 1024], F16, tag="pexp")
                nc.scalar.activation(out=pexp, in_=st, func=AF.Exp, scale=0.125)
                return pexp

            def emit_pv(h, sk, pexp, ctx):
                for j in range(2):
                    nc.tensor.matmul(
                        ctx[:, bass.ts(j, 512)],
                        Vt[:, sk, h, :],
                        pexp[:, bass.ts(j, 512)],
                        start=(sk == 0),
                        stop=(sk == NST - 1),
                    )

            def emit_normalize(qp, h, ctx):
                # copy PSUM out fast to release the ctx bank for the
                # next head; normalize from SBUF off the critical path.
                # The sums row is DMA-reshaped across 128 partitions so
                # the (8 cyc/elem) reciprocal runs 128-wide.
                q0 = qp * 1024
                po = 64 * (h % 2)
                ht = h // 2
                ctxc = small.tile([65, 1024], F32, tag="ctxc")
                nc.vector.tensor_copy(out=ctxc, in_=ctx[0:65, :])
                rr = small.tile([128, 8], F32, tag="rr")
                nc.sync.dma_start(out=rr, in_=ctxc[64:65, :])
                rrv = small.tile([128, 8], F32, tag="rrv")
                nc.vector.reciprocal(out=rrv, in_=rr)
                rinvrow = small.tile([1, 1024], F32, tag="rinvrow")
                nc.sync.dma_start(out=rinvrow, in_=rrv)
                rbc = small.tile([64, 1024], F32, tag="rbc")
                nc.gpsimd.partition_broadcast(rbc, rinvrow)
                nc.vector.tensor_mul(
                    out=ctxn[po : po + 64, ht, q0 : q0 + 1024],
                    in0=ctxc[0:64, :],
                    in1=rbc,
                )

            def emit_oproj(qp):
                # per-kt PARTIAL output projections: each (dot, j, kt) is
                # a single matmul whose ctxn[kt] slice is ready as soon as
                # head pair 2kt/2kt+1 finishes, so o-proj work backfills
                # the attention phase instead of serializing at the end.
                # The host sums the 4 kt-partials (and the 2 head-group
                # partials) in fp32.
                q0 = qp * 1024
                for kt in range(NHT):
                    for dot in range(8):
                        for j in range(2):
                            pso = ps_o.tile([128, 512], F32, tag="po")
                            nc.tensor.matmul(
                                pso[:, :],
                                wo_sb[:, kt, bass.ts(dot, 128)],
                                ctxn[:, kt, q0 + j * 512 : q0 + (j + 1) * 512],
                                start=True,
                                stop=True,
                            )
                            osb = outp.tile([128, 512], F16, tag="osb")
                            # the final head-tile's partials are the
                            # kernel tail: route their PSUM->SBUF copies
                            # through the scalar engine (idle once the
                            # exps are done) so the tail isn't DVE-serial
                            if qp == 1 and kt == NHT - 1 and (dot + j) % 2 == 0:
                                nc.scalar.copy(out=osb, in_=pso)
                            else:
                                nc.vector.tensor_copy(out=osb, in_=pso)
                            nc.sync.dma_start(
                                out=ot.ap()[kt, qp * 2 + j, dot], in_=osb
                            )

            # Projections, emitted in first-need order (program order =
            # backfill tiebreak order for the scheduler). The attention
            # stream below runs at high priority and preempts as soon as
            # each of its inputs is placed; remaining projection matmuls
            # fill the PE slack of the exp-paced attention phase.
            emit_proj("k", 0)
            nc.sync.dma_start(out=wq_sb, in_=wq.ap())
            emit_proj("q", 0, hts=(0,))
            emit_proj("q", 1, hts=(0,))
            nc.sync.dma_start(out=wv_sb, in_=wv.ap())
            emit_proj("v", 0)
            for sb in (1, 2, 3):
                # the exp chain consumes K in sb order ahead of V (PV
                # trails by the pipeline lag): let each k.ht0 group
                # leapfrog the preceding v-block in scheduler preference
                # without moving its emission position (which the xs-pool
                # WAR / in-order sync queue constrain)
                with tc.high_priority(offset=150):
                    emit_proj("k", sb, hts=(0,))
                emit_proj("v", sb)
                emit_proj("k", sb, hts=(1, 2, 3))
            emit_proj("q", 0, hts=(1, 2, 3))
            emit_proj("q", 1, hts=(1, 2, 3))
            nc.sync.dma_start(out=wo_sb, in_=wo.ap())
            emit_proj("q", 2)
            emit_proj("q", 3)

            # Attention, software-pipelined with PV lagging one iteration:
            # per slot emit scores(k+1), exp(k+1), then PV(k). The PV
            # LDWEIGHTS' waits (inherited from the matmul) are satisfied a
            # full slot early, so stationary swaps background-load under
            # the previous matmuls instead of stalling.
            iters = [
                (qp, h, sk)
                for qp in (0, 1)
                for h in range(HPC)
                for sk in range(NST)
            ]
            # PV lags its exp by TWO iterations: when the PE sequencer
            # reaches the PV pair (and the LDW/EventSemaphore carrying
            # its moved waits), the exp completed ~2.2us earlier, so the
            # sequencer flows through without stalling and the V weight
            # load hides under the preceding matmuls. (With lag-1 the
            # wait resolves exactly when the load is needed: +159ns on
            # every iteration's first PV matmul, measured.)
            LAG = 2
            with tc.high_priority():
                pendq = []  # [(qp, h, sk, pexp)]
                ctx_cur = None

                def emit_pend():
                    # PV for the oldest pending slot; allocates the
                    # head's ctx accumulator at its first write so the
                    # ps_ctx rotation dep lands after the previous
                    # head's copy-out emission.
                    nonlocal ctx_cur
                    pqp, ph, psk, ppexp = pendq.pop(0)
                    if psk == 0:
                        ctx_cur = ps_ctx.tile([128, 1024], F32, tag="ctx")
                    emit_pv(ph, psk, ppexp, ctx_cur)
                    if psk == NST - 1:
                        emit_normalize(pqp, ph, ctx_cur)

                for qp, h, sk in iters:
                    pexp = emit_scores(qp, h, sk)
                    pendq.append((qp, h, sk, pexp))
                    if len(pendq) > LAG:
                        emit_pend()
                while pendq:
                    emit_pend()

            # o-projections at normal priority: o-proj(qp0) backfills
            # attention(qp1)'s tensor slack; o-proj(qp1) is the tail.
            emit_oproj(0)
            emit_oproj(1)

    nc.compile()
    return nc


_NC_CACHE = None


def _get_nc():
    global _NC_CACHE
    if _NC_CACHE is None:
        _NC_CACHE = build_nc()
    return _NC_CACHE


def make_in_maps(q, k, v, Wq, bq, Wk, bk, Wv, bv, Wo):
    bf = np.float16

    def x_tiles(x):
        # [S, D] -> [sb, p, t, c] with s = sb*512 + c, d = t*128 + p
        return np.ascontiguousarray(
            x.reshape(NSB, 512, NDT, 128).transpose(0, 3, 2, 1)
        ).astype(bf)

    def w_tiles(w):
        # [D, DHT] -> [p, t, n] with d = t*128 + p
        return np.ascontiguousarray(
            w.reshape(NDT, 128, DHT).transpose(1, 0, 2)
        ).astype(bf)

    in_maps = []
    for core in range(N_CORES):
        b, hg = core // 2, core % 2
        csl = slice(hg * DHT, (hg + 1) * DHT)
        in_maps.append(
            {
                "xq_t": x_tiles(q[b]),
                "xk_t": x_tiles(k[b]),
                "xv_t": x_tiles(v[b]),
                "wq": w_tiles(Wq[:, csl]),
                "wk": w_tiles(Wk[:, csl]),
                "wv": w_tiles(Wv[:, csl]),
                "wo": np.ascontiguousarray(
                    Wo[csl, :].reshape(NHT, 128, D).transpose(1, 0, 2)
                ).astype(bf),
                "bq": np.ascontiguousarray(
                    bq[csl].reshape(NHT, 128).T
                ).astype(np.float32),
                "bk": np.ascontiguousarray(
                    bk[csl].reshape(NHT, 128).T
                ).astype(np.float32),
                "bv": np.ascontiguousarray(bv[csl]).astype(np.float32),
            }
        )
    return in_maps


def kernel(q, k, v, Wq, bq, Wk, bk, Wv, bv, Wo, bo):
    q = np.asarray(q, np.float32)
    k = np.asarray(k, np.float32)
    v = np.asarray(v, np.float32)
    Wq = np.asarray(Wq, np.float32)
    Wk = np.asarray(Wk, np.float32)
    Wv = np.asarray(Wv, np.float32)
    Wo = np.asarray(Wo, np.float32)
    bq = np.asarray(bq, np.float32)
    bk = np.asarray(bk, np.float32)
    bv = np.asarray(bv, np.float32)
    bo = np.asarray(bo, np.float32)

    nc = _get_nc()
    in_maps = make_in_maps(q, k, v, Wq, bq, Wk, bk, Wv, bv, Wo)
    res = run_bass_kernel_spmd(nc, in_maps, list(range(N_CORES)))
    out = np.empty((B, S, D), np.float32)
    for b in range(B):
        # o_t [kt, qb, dot, p, c]: sum kt partials (and the two
        # head-group cores), then [s, d] with s = qb*512+c, d = dot*128+p
        o_t = res.results[2 * b]["o_t"].astype(np.float32).sum(axis=0) + res.results[
            2 * b + 1
        ]["o_t"].astype(np.float32).sum(axis=0)
        out[b] = o_t.transpose(0, 3, 1, 2).reshape(S, D) + bo
    return out
